# revision 8
# baseline (speedup 1.0000x reference)
"""Causal self-attention (with the reference's inverted mask) on 8 TRN2
NeuronCores.

Problem (hardcoded): B=2, S=2048, D=1024, H=16 heads, head_dim=64, fp32.
  q/k/v = x @ W* + b*;  score = q k^T / 8;  score += tril(ones)*(-1e9)
  (inverted causal mask: the LOWER triangle incl. diagonal is masked, so
  softmax attends strictly to k > q; row q=S-1 is fully masked and its
  softmax is exactly uniform, since all its masked inputs round to exactly
  -1e9 in fp32);  out = softmax(score) @ v @ Wo + bo.

Sharding: core c handles batch b = c//4 and heads [4*(c%4), 4*(c%4)+4).
Each core computes a partial output (its 4 heads' slice of attn @ Wo);
the host sums 4 partials per batch and adds bo.

Per-core kernel (all matmuls in float32r — TF32-like, ~1.5e-4 rel err,
full PE speed at N>=256):
  Phase A: QT/KT = W^T x^T in [dh, s] layout (head pairs packed to 128
    partitions), V in [s, dh] layout with an extra ones column per head
    ([V | 1]) so one matmul later yields both the attn numerator and the
    softmax denominator.
  Phase B (per q-chunk of 512): scores computed TRANSPOSED,
    s^T[k, q] = K^T Q per (head, k-block j), so softmax needs no
    max-subtraction and no transposes of the probability matrix:
    p^T = exp(s^T/8) (safe: |s|/8 is O(10); masked entries are skipped
    or zero-filled, matching the reference where exp(-1e9 - max)
    underflows to exactly 0).  Only k-blocks j >= 4c are active; diagonal
    blocks get an affine_select zero-fill where k <= q.
    attn^T[dh|sum, q] accumulates matmul([V|1], p^T) over j in PSUM.
    The globally-masked last row (q=2047) is exactly uniform attention
    over all 2048 keys; reproduced by N=1 ones-column matmuls over all
    16 k-blocks into column 511.  Normalize via reciprocal of the sums
    row + gpsimd partition-broadcast + multiply.
  Phase C (fused per q-chunk): out_partial[s-blocks of this chunk, :] =
    attn^T.T @ Wo-rows accumulated over the 4 heads.
"""

import numpy as np

B, S, D, H, DH = 2, 2048, 1024, 16, 64
HPC = 4                 # heads per core
NCORES = 8
NPAIR = HPC // 2        # head pairs per core (2)
SBLK = S // 128         # 16 s/k blocks
NCH = S // 512          # 4 q-chunks of 512
CHUNKS = D // 128       # 8 contraction chunks of the model dim

_CACHE = {}


def _build_nc(debug=False):
    import concourse.mybir as mybir
    from concourse import bacc, tile

    f32 = mybir.dt.float32
    f32r = mybir.dt.float32r
    AF = mybir.ActivationFunctionType
    OP = mybir.AluOpType

    nc = bacc.Bacc("TRN2", target_bir_lowering=False)

    xT = nc.dram_tensor("xT", [D, S], f32, kind="ExternalInput")
    wq = nc.dram_tensor("wq", [D, HPC * DH], f32, kind="ExternalInput")
    wk = nc.dram_tensor("wk", [D, HPC * DH], f32, kind="ExternalInput")
    wv = nc.dram_tensor("wv", [D, HPC * DH], f32, kind="ExternalInput")
    wo = nc.dram_tensor("wo", [HPC * DH, D], f32, kind="ExternalInput")
    # per-pair q/k biases: [128, 4] cols = (q pair0, q pair1, k pair0, k pair1)
    bqk = nc.dram_tensor("bqk", [128, 2 * NPAIR], f32, kind="ExternalInput")
    # bv broadcast to all partitions host-side: [128, 256]
    bvb = nc.dram_tensor("bvb", [128, HPC * DH], f32, kind="ExternalInput")
    out = nc.dram_tensor("out", [S, D], f32, kind="ExternalOutput")
    if debug:
        qt_d = nc.dram_tensor("qt_d", [128, NPAIR, S], f32, kind="ExternalOutput")
        kt_d = nc.dram_tensor("kt_d", [128, NPAIR, S], f32, kind="ExternalOutput")
        vsb_d = nc.dram_tensor("vsb_d", [128, SBLK, HPC, DH + 1], f32,
                               kind="ExternalOutput")
        atn_d = nc.dram_tensor("atn_d", [64, NCH, HPC, 512], f32,
                               kind="ExternalOutput")
        psa_d = nc.dram_tensor("psa_d", [DH + 1, NCH, HPC, 512], f32,
                               kind="ExternalOutput")
        pt_d = nc.dram_tensor("pt_d", [128, NCH, HPC, 512], f32,
                              kind="ExternalOutput")

    with tile.TileContext(nc) as tc:
        with (
            tc.tile_pool(name="pers", bufs=1) as pers,
            tc.tile_pool(name="atnp", bufs=2) as atnp,
            tc.tile_pool(name="misc", bufs=1) as misc,
        ):
            qt = pers.tile([128, NPAIR, S], f32r)         # Q^T head pairs
            kt = pers.tile([128, NPAIR, S], f32r)         # K^T head pairs
            vsb = pers.tile([128, SBLK, HPC, DH + 1], f32r)  # [V | 1]
            wo_t = pers.tile([64, HPC, D], f32r)
            ones2 = misc.tile([128, 2], f32r)   # [0 | 1] columns
            onef = misc.tile([128, 2], f32)
            bias_t = misc.tile([128, 2 * NPAIR], f32)
            bvb_t = misc.tile([128, HPC * DH], f32)

            nc.sync.dma_start(bias_t[:], bqk[:])
            nc.sync.dma_start(bvb_t[:], bvb[:])
            nc.gpsimd.memset(onef[:, 0:1], 0.0)
            nc.gpsimd.memset(onef[:, 1:2], 1.0)
            nc.vector.tensor_copy(ones2[:], onef[:])  # rounded f32r [0|1]
            # ones column of [V|1] for every (sblk, head)
            nc.vector.tensor_copy(
                vsb[:, :, :, DH:DH + 1],
                onef[:, 1:2].to_broadcast((128, SBLK, HPC, 1)))

            # ---------------- Phase A: projections ----------------
            with (
                tc.tile_pool(name="stw", bufs=1) as stwp,
                tc.tile_pool(name="stx", bufs=2) as stxp,
                tc.tile_pool(name="wts", bufs=1) as wts,
                tc.tile_pool(name="psA", bufs=4, space="PSUM") as psA,
                tc.tile_pool(name="psV", bufs=2, space="PSUM") as psV,
            ):
                xtr = wts.tile([128, CHUNKS, S], f32r)
                wq_t = wts.tile([128, CHUNKS, HPC * DH], f32r, tag="wq")
                wk_t = wts.tile([128, CHUNKS, HPC * DH], f32r, tag="wk")
                wv_t = wts.tile([128, CHUNKS, HPC * DH], f32r, tag="wv")

                for w_dram, w_tile in ((wq, wq_t), (wk, wk_t), (wv, wv_t)):
                    st = stwp.tile([128, CHUNKS, HPC * DH], f32, tag="stw")
                    nc.sync.dma_start(
                        st[:], w_dram.rearrange("(c p) m -> p c m", p=128))
                    nc.vector.tensor_copy(w_tile[:], st[:])
                # Wo in two halves through the same staging slot
                wo_r = wo.rearrange("(h d) n -> d h n", d=64)
                for hh in range(2):
                    st = stwp.tile([64, 2, D], f32, tag="stw")
                    nc.sync.dma_start(st[:], wo_r[:, 2 * hh:2 * hh + 2, :])
                    nc.vector.tensor_copy(wo_t[:, 2 * hh:2 * hh + 2, :], st[:])

                xT_r = xT.rearrange("(c p) s -> c p s", p=128)
                for c in range(CHUNKS):
                    for half in range(2):
                        st = stxp.tile([128, S // 2], f32, tag="stx")
                        sl = slice(half * (S // 2), (half + 1) * (S // 2))
                        nc.sync.dma_start(st[:], xT_r[c][:, sl])
                        nc.vector.tensor_copy(xtr[:, c, sl], st[:])

                # QT / KT: psum[128(2xdh), 512] accumulated over chunks
                for w_tile, dst, bcol0 in ((wq_t, qt, 0), (wk_t, kt, NPAIR)):
                    for p in range(NPAIR):
                        for n in range(NCH):
                            ps = psA.tile([128, 512], f32)
                            for c in range(CHUNKS):
                                nc.tensor.matmul(
                                    ps[:],
                                    w_tile[:, c, 128 * p:128 * p + 128],
                                    xtr[:, c, 512 * n:512 * n + 512],
                                    start=(c == 0), stop=(c == CHUNKS - 1))
                            # evacuate + add per-partition bias (dh rows)
                            nc.scalar.activation(
                                dst[:, p, 512 * n:512 * n + 512], ps[:],
                                AF.Identity,
                                bias=bias_t[:, bcol0 + p:bcol0 + p + 1])

                # V: psum[128(s), 256] accumulated over chunks
                for sb in range(SBLK):
                    ps = psV.tile([128, HPC * DH], f32)
                    for c in range(CHUNKS):
                        nc.tensor.matmul(
                            ps[:],
                            xtr[:, c, 128 * sb:128 * sb + 128],
                            wv_t[:, c, :],
                            start=(c == 0), stop=(c == CHUNKS - 1))
                    nc.vector.tensor_tensor(
                        vsb[:, sb, :, 0:DH],
                        ps[:].rearrange("p (h d) -> p h d", h=HPC),
                        bvb_t[:].rearrange("p (h d) -> p h d", h=HPC),
                        op=OP.add)
                if debug:
                    nc.sync.dma_start(qt_d[:], qt[:].bitcast(f32))
                    nc.sync.dma_start(kt_d[:], kt[:].bitcast(f32))
                    nc.sync.dma_start(vsb_d[:], vsb[:].bitcast(f32))

            # ------------- Phase B + fused C, per q-chunk -------------
            with (
                tc.tile_pool(name="pt", bufs=6) as ptp,
                tc.tile_pool(name="rec", bufs=4) as recp,
                tc.tile_pool(name="bc", bufs=4) as bcp,
                tc.tile_pool(name="ob", bufs=4) as obp,
                tc.tile_pool(name="psS", bufs=2, space="PSUM") as psS,
                tc.tile_pool(name="psAt", bufs=1, space="PSUM") as psAt,
                tc.tile_pool(name="psO", bufs=2, space="PSUM") as psO,
            ):
                for ch in range(NCH):
                    js = list(range(4 * ch, SBLK))
                    psa = [psAt.tile([DH + 1, 512], f32, tag=f"psa{h}",
                                     name=f"psa{h}")
                           for h in range(HPC)]
                    for idx, j in enumerate(js):
                        pts = []
                        for h in range(HPC):
                            pair, half = h // 2, h % 2
                            r0 = 64 * half
                            pss = psS.tile([128, 512], f32)
                            nc.tensor.matmul(
                                pss[:],
                                kt[r0:r0 + 64, pair, 128 * j:128 * j + 128],
                                qt[r0:r0 + 64, pair, 512 * ch:512 * ch + 512],
                                start=True, stop=True)
                            pt = ptp.tile([128, 512], f32r)
                            nc.scalar.activation(pt[:], pss[:], AF.Exp,
                                                 scale=0.125)
                            if 4 * ch <= j <= 4 * ch + 3:
                                # zero-fill where k <= q (inverted causal)
                                nc.gpsimd.affine_select(
                                    pt[:], pt[:],
                                    pattern=[[-1, 512]],
                                    base=128 * j - 512 * ch,
                                    channel_multiplier=1,
                                    compare_op=OP.is_gt,
                                    fill=0.0)
                            if debug and j == 4 * ch:
                                nc.sync.dma_start(
                                    pt_d[:, ch, h, :], pt[:].bitcast(f32))
                            pts.append(pt)
                        last = (idx == len(js) - 1) and ch < 3
                        for h in range(HPC):
                            nc.tensor.matmul(
                                psa[h][:], vsb[:, j, h, :], pts[h][:],
                                start=(idx == 0), stop=last)
                    if ch == 3:
                        # last global row q=2047: uniform over ALL keys.
                        # Column 511 is all-zero after masking; accumulate
                        # sum_k V[k] and the count 2048 via ones columns.
                        for j in range(SBLK):
                            for h in range(HPC):
                                nc.tensor.matmul(
                                    psa[h][:, 510:512],
                                    vsb[:, j, h, :], ones2[:],
                                    start=False,
                                    stop=(j == SBLK - 1))
                    # normalize: attn^T rows / sums row
                    atn = atnp.tile([64, HPC, 512], f32r)
                    for h in range(HPC):
                        rec = recp.tile([DH + 1, 512], f32)
                        nc.vector.reciprocal(rec[DH:DH + 1, :],
                                             psa[h][DH:DH + 1, :])
                        # hw partition_broadcast only reads partition 0:
                        # DMA-shift the reciprocal row from partition 64 to 0
                        rec0 = recp.tile([1, 512], f32, name="rec0",
                                         tag="rec0")
                        nc.sync.dma_start(rec0[0:1, :], rec[DH:DH + 1, :])
                        bc = bcp.tile([64, 512], f32)
                        nc.gpsimd.partition_broadcast(bc[:], rec0[0:1, :])
                        if debug:
                            dcp = recp.tile([DH + 1, 512], f32, name="dcp")
                            nc.vector.tensor_copy(dcp[:], psa[h][:])
                            nc.sync.dma_start(psa_d[:, ch, h, :], dcp[:])
                        nc.vector.tensor_tensor(
                            atn[:, h, :], psa[h][0:DH, :], bc[:], op=OP.mult)
                    if debug:
                        nc.sync.dma_start(
                            atn_d[:, ch, :, :],
                            atn[:].bitcast(f32))

                    # fused phase C for this chunk's 4 s-blocks
                    for k in range(4):
                        sb = 4 * ch + k
                        for n in range(2):
                            ps = psO.tile([128, 512], f32)
                            for h in range(HPC):
                                nc.tensor.matmul(
                                    ps[:],
                                    atn[:, h, 128 * k:128 * k + 128],
                                    wo_t[:, h, 512 * n:512 * n + 512],
                                    start=(h == 0), stop=(h == HPC - 1))
                            ob = obp.tile([128, 512], f32)
                            nc.vector.tensor_copy(ob[:], ps[:])
                            nc.sync.dma_start(
                                out[128 * sb:128 * sb + 128,
                                    512 * n:512 * n + 512], ob[:])

    nc.finalize()
    return nc


def _prep_in_maps(inputs, Wq, bq, Wk, bk, Wv, bv, Wo, bo):
    in_maps = []
    xTs = [np.ascontiguousarray(inputs[b].T) for b in range(B)]
    for core in range(NCORES):
        b = core // (NCORES // B)
        g = core % (NCORES // B)
        cols = slice(g * HPC * DH, (g + 1) * HPC * DH)
        bq_c = bq[cols].reshape(NPAIR, 128).T          # [128, 2]
        bk_c = bk[cols].reshape(NPAIR, 128).T
        bqk_c = np.ascontiguousarray(
            np.concatenate([bq_c, bk_c], axis=1), dtype=np.float32)
        bvb_c = np.ascontiguousarray(
            np.broadcast_to(bv[cols][None, :], (128, HPC * DH)),
            dtype=np.float32)
        in_maps.append({
            "xT": xTs[b],
            "wq": np.ascontiguousarray(Wq[:, cols]),
            "wk": np.ascontiguousarray(Wk[:, cols]),
            "wv": np.ascontiguousarray(Wv[:, cols]),
            "wo": np.ascontiguousarray(Wo[cols, :]),
            "bqk": bqk_c,
            "bvb": bvb_c,
        })
    return in_maps


def kernel(inputs, Wq, bq, Wk, bk, Wv, bv, Wo, bo, _want_results=False,
           **_run_kwargs):
    from concourse.bass_utils import run_bass_kernel_spmd

    inputs = np.asarray(inputs, dtype=np.float32)
    Wq, bq = np.asarray(Wq, np.float32), np.asarray(bq, np.float32)
    Wk, bk = np.asarray(Wk, np.float32), np.asarray(bk, np.float32)
    Wv, bv = np.asarray(Wv, np.float32), np.asarray(bv, np.float32)
    Wo, bo = np.asarray(Wo, np.float32), np.asarray(bo, np.float32)

    if "nc" not in _CACHE:
        _CACHE["nc"] = _build_nc()
    nc = _CACHE["nc"]

    in_maps = _prep_in_maps(inputs, Wq, bq, Wk, bk, Wv, bv, Wo, bo)
    res = run_bass_kernel_spmd(nc, in_maps, core_ids=list(range(NCORES)),
                               **_run_kwargs)

    out = np.zeros((B, S, D), dtype=np.float32)
    for core in range(NCORES):
        b = core // (NCORES // B)
        out[b] += res.results[core]["out"]
    out += bo[None, None, :]
    if _want_results:
        return out, res
    return out


# revision 10
# speedup vs baseline: 1.1140x; 1.1140x over previous
"""Causal self-attention (with the reference's inverted mask) on 8 TRN2
NeuronCores.

Problem (hardcoded): B=2, S=2048, D=1024, H=16 heads, head_dim=64, fp32.
  q/k/v = x @ W* + b*;  score = q k^T / 8;  score += tril(ones)*(-1e9)
  (inverted causal mask: the LOWER triangle incl. diagonal is masked, so
  softmax attends strictly to k > q; row q=S-1 is fully masked and its
  softmax is exactly uniform, since all its masked inputs round to exactly
  -1e9 in fp32);  out = softmax(score) @ v @ Wo + bo.

Sharding: core c handles batch b = c//4 and heads [4*(c%4), 4*(c%4)+4).
Each core computes a partial output (its 4 heads' slice of attn @ Wo);
the host sums 4 partials per batch and adds bo.

Per-core kernel (all matmuls in float32r — TF32-like, ~1.5e-4 rel err,
full PE speed at N>=256):
  Phase A: QT/KT = W^T x^T in [dh, s] layout (head pairs packed to 128
    partitions), V in [s, dh] layout with an extra ones column per head
    ([V | 1]) so one matmul later yields both the attn numerator and the
    softmax denominator.
  Phase B (per q-chunk of 512): scores computed TRANSPOSED,
    s^T[k, q] = K^T Q per (head, k-block j), so softmax needs no
    max-subtraction and no transposes of the probability matrix:
    p^T = exp(s^T/8) (safe: |s|/8 is O(10); masked entries are skipped
    or zero-filled, matching the reference where exp(-1e9 - max)
    underflows to exactly 0).  Only k-blocks j >= 4c are active, and for
    diagonal blocks j = 4c+d only the first 128(d+1) q-columns can be
    unmasked, so score/exp/select/attn all narrow to that width; the
    in-block triangle gets an affine_select zero-fill where k <= q.
    attn^T[dh|sum, q] accumulates matmul([V|1], p^T) over j in PSUM.
    The globally-masked last row (q=2047) is exactly uniform attention
    over all 2048 keys; reproduced by N=2 [0|1]-column matmuls over all
    16 k-blocks into columns 510:512 (adding zero to 510).
    Normalization: broadcast the sums row to 64 partitions with a K=1
    ones matmul, then a 64-lane reciprocal and multiply (a 1-lane
    reciprocal measured 3.3us; this path is ~10x cheaper).
  Phase C (fused per q-chunk): out_partial[s-blocks of this chunk, :] =
    attn^T.T @ Wo-rows, heads packed in pairs so the contraction runs
    K=128 (odd heads DMA-shifted to partitions 64:128).
"""

import numpy as np

B, S, D, H, DH = 2, 2048, 1024, 16, 64
HPC = 4                 # heads per core
NCORES = 8
NPAIR = HPC // 2        # head pairs per core (2)
SBLK = S // 128         # 16 s/k blocks
NCH = S // 512          # 4 q-chunks of 512
CHUNKS = D // 128       # 8 contraction chunks of the model dim

_CACHE = {}


def _build_nc(debug=False):
    import concourse.mybir as mybir
    from concourse import bacc, tile

    f32 = mybir.dt.float32
    f32r = mybir.dt.float32r
    AF = mybir.ActivationFunctionType
    OP = mybir.AluOpType

    nc = bacc.Bacc("TRN2", target_bir_lowering=False)

    xT = nc.dram_tensor("xT", [D, S], f32, kind="ExternalInput")
    wq = nc.dram_tensor("wq", [D, HPC * DH], f32, kind="ExternalInput")
    wk = nc.dram_tensor("wk", [D, HPC * DH], f32, kind="ExternalInput")
    wv = nc.dram_tensor("wv", [D, HPC * DH], f32, kind="ExternalInput")
    wo = nc.dram_tensor("wo", [HPC * DH, D], f32, kind="ExternalInput")
    # per-pair q/k biases: [128, 4] cols = (q pair0, q pair1, k pair0, k pair1)
    bqk = nc.dram_tensor("bqk", [128, 2 * NPAIR], f32, kind="ExternalInput")
    # bv broadcast to all partitions host-side: [128, 256]
    bvb = nc.dram_tensor("bvb", [128, HPC * DH], f32, kind="ExternalInput")
    out = nc.dram_tensor("out", [S, D], f32, kind="ExternalOutput")
    if debug:
        qt_d = nc.dram_tensor("qt_d", [128, NPAIR, S], f32,
                              kind="ExternalOutput")
        kt_d = nc.dram_tensor("kt_d", [128, NPAIR, S], f32,
                              kind="ExternalOutput")
        vsb_d = nc.dram_tensor("vsb_d", [128, SBLK, HPC, DH + 1], f32,
                               kind="ExternalOutput")
        atn_d = nc.dram_tensor("atn_d", [128, NCH, NPAIR, 512], f32,
                               kind="ExternalOutput")
        psa_d = nc.dram_tensor("psa_d", [DH + 1, NCH, HPC, 512], f32,
                               kind="ExternalOutput")

    with tile.TileContext(nc) as tc:
        with (
            tc.tile_pool(name="pers", bufs=1) as pers,
            tc.tile_pool(name="atnp", bufs=2) as atnp,
            tc.tile_pool(name="misc", bufs=1) as misc,
        ):
            qt = pers.tile([128, NPAIR, S], f32r)         # Q^T head pairs
            kt = pers.tile([128, NPAIR, S], f32r)         # K^T head pairs
            vsb = pers.tile([128, SBLK, HPC, DH + 1], f32r)  # [V | 1]
            wo_t = pers.tile([128, NPAIR, D], f32r)       # Wo head pairs
            ones2 = misc.tile([128, 2], f32r)   # [0 | 1] columns
            onef = misc.tile([128, 2], f32)
            onesrow = misc.tile([DH + 1, DH], f32r)  # row 64 = ones
            bias_t = misc.tile([128, 2 * NPAIR], f32)
            bvb_t = misc.tile([128, HPC * DH], f32)

            nc.sync.dma_start(bias_t[:], bqk[:])
            nc.sync.dma_start(bvb_t[:], bvb[:])
            nc.gpsimd.memset(onef[:, 0:1], 0.0)
            nc.gpsimd.memset(onef[:, 1:2], 1.0)
            nc.vector.tensor_copy(ones2[:], onef[:])  # rounded f32r [0|1]
            nc.vector.tensor_copy(
                onesrow[DH:DH + 1, :],
                onef[DH:DH + 1, 1:2].to_broadcast((1, DH)))
            # ones column of [V|1] for every (sblk, head)
            nc.vector.tensor_copy(
                vsb[:, :, :, DH:DH + 1],
                onef[:, 1:2].to_broadcast((128, SBLK, HPC, 1)))

            # ---------------- Phase A: projections ----------------
            with (
                tc.tile_pool(name="stw", bufs=1) as stwp,
                tc.tile_pool(name="stx", bufs=2) as stxp,
                tc.tile_pool(name="wts", bufs=1) as wts,
                tc.tile_pool(name="psA", bufs=4, space="PSUM") as psA,
                tc.tile_pool(name="psV", bufs=2, space="PSUM") as psV,
            ):
                xtr = wts.tile([128, CHUNKS, S], f32r)
                wq_t = wts.tile([128, CHUNKS, HPC * DH], f32r, tag="wq")
                wk_t = wts.tile([128, CHUNKS, HPC * DH], f32r, tag="wk")
                wv_t = wts.tile([128, CHUNKS, HPC * DH], f32r, tag="wv")

                for w_dram, w_tile in ((wq, wq_t), (wk, wk_t), (wv, wv_t)):
                    st = stwp.tile([128, CHUNKS, HPC * DH], f32, tag="stw")
                    nc.sync.dma_start(
                        st[:], w_dram.rearrange("(c p) m -> p c m", p=128))
                    nc.vector.tensor_copy(w_tile[:], st[:])
                # Wo pairs: rows of pair p = wo[128p : 128p+128]
                wo_r = wo.rearrange("(p r) n -> p r n", r=128)
                for p in range(NPAIR):
                    st = stwp.tile([128, D], f32, tag="stw")
                    nc.sync.dma_start(st[:], wo_r[p])
                    nc.vector.tensor_copy(wo_t[:, p, :], st[:])

                xT_r = xT.rearrange("(c p) s -> c p s", p=128)
                for c in range(CHUNKS):
                    for half in range(2):
                        st = stxp.tile([128, S // 2], f32, tag="stx")
                        sl = slice(half * (S // 2), (half + 1) * (S // 2))
                        nc.sync.dma_start(st[:], xT_r[c][:, sl])
                        nc.vector.tensor_copy(xtr[:, c, sl], st[:])

                # QT / KT: psum[128(2xdh), 512] accumulated over chunks
                for w_tile, dst, bcol0 in ((wq_t, qt, 0), (wk_t, kt, NPAIR)):
                    for p in range(NPAIR):
                        for n in range(NCH):
                            ps = psA.tile([128, 512], f32)
                            for c in range(CHUNKS):
                                nc.tensor.matmul(
                                    ps[:],
                                    w_tile[:, c, 128 * p:128 * p + 128],
                                    xtr[:, c, 512 * n:512 * n + 512],
                                    start=(c == 0), stop=(c == CHUNKS - 1))
                            # evacuate + add per-partition bias (dh rows)
                            nc.scalar.activation(
                                dst[:, p, 512 * n:512 * n + 512], ps[:],
                                AF.Identity,
                                bias=bias_t[:, bcol0 + p:bcol0 + p + 1])

                # V: psum[128(s), 256] accumulated over chunks
                for sb in range(SBLK):
                    ps = psV.tile([128, HPC * DH], f32)
                    for c in range(CHUNKS):
                        nc.tensor.matmul(
                            ps[:],
                            xtr[:, c, 128 * sb:128 * sb + 128],
                            wv_t[:, c, :],
                            start=(c == 0), stop=(c == CHUNKS - 1))
                    nc.vector.tensor_tensor(
                        vsb[:, sb, :, 0:DH],
                        ps[:].rearrange("p (h d) -> p h d", h=HPC),
                        bvb_t[:].rearrange("p (h d) -> p h d", h=HPC),
                        op=OP.add)
                if debug:
                    nc.sync.dma_start(qt_d[:], qt[:].bitcast(f32))
                    nc.sync.dma_start(kt_d[:], kt[:].bitcast(f32))
                    nc.sync.dma_start(vsb_d[:], vsb[:].bitcast(f32))

            # ------------- Phase B + fused C, per q-chunk -------------
            with (
                tc.tile_pool(name="pt", bufs=6) as ptp,
                tc.tile_pool(name="srow", bufs=2) as srowp,
                tc.tile_pool(name="rcp", bufs=2) as rcpp,
                tc.tile_pool(name="todd", bufs=2) as toddp,
                tc.tile_pool(name="ob", bufs=4) as obp,
                tc.tile_pool(name="psS", bufs=2, space="PSUM") as psS,
                tc.tile_pool(name="psAt", bufs=1, space="PSUM") as psAt,
                tc.tile_pool(name="psO", bufs=2, space="PSUM") as psO,
            ):
                for ch in range(NCH):
                    js = list(range(4 * ch, SBLK))
                    psa = [psAt.tile([DH + 1, 512], f32, tag=f"psa{h}",
                                     name=f"psa{h}")
                           for h in range(HPC)]
                    for idx, j in enumerate(js):
                        d = j - 4 * ch
                        W = 128 * (d + 1) if d < 4 else 512
                        pts = []
                        for h in range(HPC):
                            pair, half = h // 2, h % 2
                            r0 = 64 * half
                            pss = psS.tile([128, 512], f32, tag="pss",
                                           name="pss")
                            nc.tensor.matmul(
                                pss[:, 0:W],
                                kt[r0:r0 + 64, pair, 128 * j:128 * j + 128],
                                qt[r0:r0 + 64, pair,
                                   512 * ch:512 * ch + W],
                                start=True, stop=True)
                            pt = ptp.tile([128, 512], f32r)
                            nc.scalar.activation(pt[:, 0:W], pss[:, 0:W],
                                                 AF.Exp, scale=0.125)
                            if d < 4:
                                # zero-fill where k <= q (inverted causal)
                                nc.gpsimd.affine_select(
                                    pt[:, 0:W], pt[:, 0:W],
                                    pattern=[[-1, W]],
                                    base=128 * j - 512 * ch,
                                    channel_multiplier=1,
                                    compare_op=OP.is_gt,
                                    fill=0.0)
                            pts.append(pt)
                        last = (idx == len(js) - 1) and ch < 3
                        for h in range(HPC):
                            nc.tensor.matmul(
                                psa[h][:, 0:W], vsb[:, j, h, :],
                                pts[h][:, 0:W],
                                start=(idx == 0), stop=last)
                    if ch == 3:
                        # last global row q=2047: uniform over ALL keys.
                        # Column 511 is all-zero after masking; accumulate
                        # sum_k V[k] and the count 2048 via [0|1] columns.
                        for j in range(SBLK):
                            for h in range(HPC):
                                nc.tensor.matmul(
                                    psa[h][:, 510:512],
                                    vsb[:, j, h, :], ones2[:],
                                    start=False,
                                    stop=(j == SBLK - 1))
                    # normalize: attn^T rows / sums row.  Broadcast the
                    # sums row via a K=1 ones matmul, 64-lane reciprocal,
                    # then multiply.  Odd heads go through a SBUF tile and
                    # a partition-shifting DMA into rows 64:128 of the
                    # pair tile so phase C can contract K=128.
                    atn = atnp.tile([128, NPAIR, 512], f32r)
                    for h in range(HPC):
                        pair, half = h // 2, h % 2
                        srow = srowp.tile([DH + 1, 512], f32r)
                        nc.scalar.copy(srow[DH:DH + 1, :],
                                       psa[h][DH:DH + 1, :])
                        bcs = psS.tile([64, 512], f32, tag="pss",
                                       name="bcs")
                        nc.tensor.matmul(bcs[:],
                                         onesrow[DH:DH + 1, :],
                                         srow[DH:DH + 1, :],
                                         start=True, stop=True)
                        rcp = rcpp.tile([64, 512], f32)
                        nc.vector.reciprocal(rcp[:], bcs[:])
                        if half == 0:
                            nc.vector.tensor_tensor(
                                atn[0:64, pair, :], psa[h][0:DH, :],
                                rcp[:], op=OP.mult)
                        else:
                            todd = toddp.tile([64, 512], f32r)
                            nc.vector.tensor_tensor(
                                todd[:], psa[h][0:DH, :], rcp[:],
                                op=OP.mult)
                            nc.sync.dma_start(atn[64:128, pair, :],
                                              todd[:])
                    if debug:
                        for h in range(HPC):
                            dcp = srowp.tile([DH + 1, 512], f32,
                                             name="dcp", tag="dcp")
                            nc.vector.tensor_copy(dcp[:], psa[h][:])
                            nc.sync.dma_start(psa_d[:, ch, h, :], dcp[:])
                        nc.sync.dma_start(atn_d[:, ch, :, :],
                                          atn[:].bitcast(f32))

                    # fused phase C for this chunk's 4 s-blocks
                    for k in range(4):
                        sb = 4 * ch + k
                        for n in range(2):
                            ps = psO.tile([128, 512], f32)
                            for p in range(NPAIR):
                                nc.tensor.matmul(
                                    ps[:],
                                    atn[:, p, 128 * k:128 * k + 128],
                                    wo_t[:, p, 512 * n:512 * n + 512],
                                    start=(p == 0), stop=(p == NPAIR - 1))
                            ob = obp.tile([128, 512], f32)
                            nc.vector.tensor_copy(ob[:], ps[:])
                            nc.sync.dma_start(
                                out[128 * sb:128 * sb + 128,
                                    512 * n:512 * n + 512], ob[:])

    nc.finalize()
    return nc


def _prep_in_maps(inputs, Wq, bq, Wk, bk, Wv, bv, Wo, bo):
    in_maps = []
    xTs = [np.ascontiguousarray(inputs[b].T) for b in range(B)]
    for core in range(NCORES):
        b = core // (NCORES // B)
        g = core % (NCORES // B)
        cols = slice(g * HPC * DH, (g + 1) * HPC * DH)
        bq_c = bq[cols].reshape(NPAIR, 128).T          # [128, 2]
        bk_c = bk[cols].reshape(NPAIR, 128).T
        bqk_c = np.ascontiguousarray(
            np.concatenate([bq_c, bk_c], axis=1), dtype=np.float32)
        bvb_c = np.ascontiguousarray(
            np.broadcast_to(bv[cols][None, :], (128, HPC * DH)),
            dtype=np.float32)
        in_maps.append({
            "xT": xTs[b],
            "wq": np.ascontiguousarray(Wq[:, cols]),
            "wk": np.ascontiguousarray(Wk[:, cols]),
            "wv": np.ascontiguousarray(Wv[:, cols]),
            "wo": np.ascontiguousarray(Wo[cols, :]),
            "bqk": bqk_c,
            "bvb": bvb_c,
        })
    return in_maps


def kernel(inputs, Wq, bq, Wk, bk, Wv, bv, Wo, bo, _want_results=False,
           **_run_kwargs):
    from concourse.bass_utils import run_bass_kernel_spmd

    inputs = np.asarray(inputs, dtype=np.float32)
    Wq, bq = np.asarray(Wq, np.float32), np.asarray(bq, np.float32)
    Wk, bk = np.asarray(Wk, np.float32), np.asarray(bk, np.float32)
    Wv, bv = np.asarray(Wv, np.float32), np.asarray(bv, np.float32)
    Wo, bo = np.asarray(Wo, np.float32), np.asarray(bo, np.float32)

    if "nc" not in _CACHE:
        _CACHE["nc"] = _build_nc()
    nc = _CACHE["nc"]

    in_maps = _prep_in_maps(inputs, Wq, bq, Wk, bk, Wv, bv, Wo, bo)
    res = run_bass_kernel_spmd(nc, in_maps, core_ids=list(range(NCORES)),
                               **_run_kwargs)

    out = np.zeros((B, S, D), dtype=np.float32)
    for core in range(NCORES):
        b = core // (NCORES // B)
        out[b] += res.results[core]["out"]
    out += bo[None, None, :]
    if _want_results:
        return out, res
    return out


# revision 11
# speedup vs baseline: 1.2201x; 1.0953x over previous
"""Causal self-attention (with the reference's inverted mask) on 8 TRN2
NeuronCores.

Problem (hardcoded): B=2, S=2048, D=1024, H=16 heads, head_dim=64, fp32.
  q/k/v = x @ W* + b*;  score = q k^T / 8;  score += tril(ones)*(-1e9)
  (inverted causal mask: the LOWER triangle incl. diagonal is masked, so
  softmax attends strictly to k > q; row q=S-1 is fully masked and its
  softmax is exactly uniform, since all its masked inputs round to exactly
  -1e9 in fp32);  out = softmax(score) @ v @ Wo + bo.

Sharding: core c handles batch b = c//4 and heads [4*(c%4), 4*(c%4)+4).
Each core computes a partial output (its 4 heads' slice of attn @ Wo);
the host sums 4 partials per batch and adds bo.

Per-core kernel (all matmuls in float32r — TF32-like, ~1.5e-4 rel err,
full PE speed at N>=256):
  Phase A: QT/KT = W^T x^T in [dh, s] layout (head pairs packed to 128
    partitions), V in [s, dh] layout with an extra ones column per head
    ([V | 1]) so one matmul later yields both the attn numerator and the
    softmax denominator.
  Phase B (per q-chunk of 512): scores computed TRANSPOSED,
    s^T[k, q] = K^T Q per (head, k-block j), so softmax needs no
    max-subtraction and no transposes of the probability matrix:
    p^T = exp(s^T/8) (safe: |s|/8 is O(10); masked entries are skipped
    or zero-filled, matching the reference where exp(-1e9 - max)
    underflows to exactly 0).  Only k-blocks j >= 4c are active, and for
    diagonal blocks j = 4c+d only the first 128(d+1) q-columns can be
    unmasked, so score/exp/select/attn all narrow to that width; the
    in-block triangle gets an affine_select zero-fill where k <= q.
    attn^T[dh|sum, q] accumulates matmul([V|1], p^T) over j in PSUM.
    The globally-masked last row (q=2047) is exactly uniform attention
    over all 2048 keys; reproduced by N=2 [0|1]-column matmuls over all
    16 k-blocks into columns 510:512 (adding zero to 510).
    Normalization: broadcast the sums row to 64 partitions with a K=1
    ones matmul, then a 64-lane reciprocal and multiply (a 1-lane
    reciprocal measured 3.3us; this path is ~10x cheaper).
  Phase C (fused per q-chunk): out_partial[s-blocks of this chunk, :] =
    attn^T.T @ Wo-rows, heads packed in pairs so the contraction runs
    K=128 (odd heads DMA-shifted to partitions 64:128).
"""

import numpy as np

B, S, D, H, DH = 2, 2048, 1024, 16, 64
HPC = 4                 # heads per core
NCORES = 8
NPAIR = HPC // 2        # head pairs per core (2)
SBLK = S // 128         # 16 s/k blocks
NCH = S // 512          # 4 q-chunks of 512
CHUNKS = D // 128       # 8 contraction chunks of the model dim

_CACHE = {}


def _build_nc(debug=False):
    import concourse.mybir as mybir
    from concourse import bacc, tile

    f32 = mybir.dt.float32
    f32r = mybir.dt.float32r
    AF = mybir.ActivationFunctionType
    OP = mybir.AluOpType

    nc = bacc.Bacc("TRN2", target_bir_lowering=False)

    xT = nc.dram_tensor("xT", [D, S], f32, kind="ExternalInput")
    wq = nc.dram_tensor("wq", [D, HPC * DH], f32, kind="ExternalInput")
    wk = nc.dram_tensor("wk", [D, HPC * DH], f32, kind="ExternalInput")
    wv = nc.dram_tensor("wv", [D, HPC * DH], f32, kind="ExternalInput")
    wo = nc.dram_tensor("wo", [HPC * DH, D], f32, kind="ExternalInput")
    # per-pair q/k biases: [128, 4] cols = (q pair0, q pair1, k pair0, k pair1)
    bqk = nc.dram_tensor("bqk", [128, 2 * NPAIR], f32, kind="ExternalInput")
    # bv broadcast to all partitions host-side: [128, 256]
    bvb = nc.dram_tensor("bvb", [128, HPC * DH], f32, kind="ExternalInput")
    out = nc.dram_tensor("out", [S, D], f32, kind="ExternalOutput")
    if debug:
        qt_d = nc.dram_tensor("qt_d", [128, NPAIR, S], f32,
                              kind="ExternalOutput")
        kt_d = nc.dram_tensor("kt_d", [128, NPAIR, S], f32,
                              kind="ExternalOutput")
        vsb_d = nc.dram_tensor("vsb_d", [128, SBLK, HPC, DH + 1], f32,
                               kind="ExternalOutput")
        atn_d = nc.dram_tensor("atn_d", [128, NCH, NPAIR, 512], f32,
                               kind="ExternalOutput")
        psa_d = nc.dram_tensor("psa_d", [DH + 1, NCH, HPC, 512], f32,
                               kind="ExternalOutput")

    with tile.TileContext(nc) as tc:
        with (
            tc.tile_pool(name="pers", bufs=1) as pers,
            tc.tile_pool(name="atnp", bufs=2) as atnp,
            tc.tile_pool(name="misc", bufs=1) as misc,
        ):
            qt = pers.tile([128, NPAIR, S], f32r)         # Q^T head pairs
            kt = pers.tile([128, NPAIR, S], f32r)         # K^T head pairs
            vsb = pers.tile([128, SBLK, HPC, DH + 1], f32r)  # [V | 1]
            wo_t = pers.tile([128, NPAIR, D], f32r)       # Wo head pairs
            ones2 = misc.tile([128, 2], f32r)   # [0 | 1] columns
            onef = misc.tile([128, 2], f32)
            onesrow = misc.tile([DH + 1, DH], f32r)  # row 64 = ones
            bias_t = misc.tile([128, 2 * NPAIR], f32)
            bvb_t = misc.tile([128, HPC * DH], f32)

            nc.sync.dma_start(bias_t[:], bqk[:])
            nc.sync.dma_start(bvb_t[:], bvb[:])
            nc.gpsimd.memset(onef[:, 0:1], 0.0)
            nc.gpsimd.memset(onef[:, 1:2], 1.0)
            nc.vector.tensor_copy(ones2[:], onef[:])  # rounded f32r [0|1]
            nc.vector.tensor_copy(
                onesrow[DH:DH + 1, :],
                onef[DH:DH + 1, 1:2].to_broadcast((1, DH)))
            # ones column of [V|1] for every (sblk, head)
            nc.vector.tensor_copy(
                vsb[:, :, :, DH:DH + 1],
                onef[:, 1:2].to_broadcast((128, SBLK, HPC, 1)))

            # ---------------- Phase A: projections ----------------
            with (
                tc.tile_pool(name="stw", bufs=1) as stwp,
                tc.tile_pool(name="stx", bufs=2) as stxp,
                tc.tile_pool(name="wts", bufs=1) as wts,
                tc.tile_pool(name="psA", bufs=4, space="PSUM") as psA,
                tc.tile_pool(name="psV", bufs=2, space="PSUM") as psV,
            ):
                xtr = wts.tile([128, CHUNKS, S], f32r)
                wq_t = wts.tile([128, CHUNKS, HPC * DH], f32r, tag="wq")
                wk_t = wts.tile([128, CHUNKS, HPC * DH], f32r, tag="wk")
                wv_t = wts.tile([128, CHUNKS, HPC * DH], f32r, tag="wv")

                for w_dram, w_tile in ((wq, wq_t), (wk, wk_t), (wv, wv_t)):
                    st = stwp.tile([128, CHUNKS, HPC * DH], f32, tag="stw")
                    nc.sync.dma_start(
                        st[:], w_dram.rearrange("(c p) m -> p c m", p=128))
                    nc.vector.tensor_copy(w_tile[:], st[:])
                # Wo pairs: rows of pair p = wo[128p : 128p+128]
                wo_r = wo.rearrange("(p r) n -> p r n", r=128)
                for p in range(NPAIR):
                    st = stwp.tile([128, D], f32, tag="stw")
                    nc.sync.dma_start(st[:], wo_r[p])
                    nc.vector.tensor_copy(wo_t[:, p, :], st[:])

                xT_r = xT.rearrange("(c p) s -> c p s", p=128)
                for c in range(CHUNKS):
                    for half in range(2):
                        st = stxp.tile([128, S // 2], f32, tag="stx")
                        sl = slice(half * (S // 2), (half + 1) * (S // 2))
                        nc.sync.dma_start(st[:], xT_r[c][:, sl])
                        nc.vector.tensor_copy(xtr[:, c, sl], st[:])

                # QT / KT: psum[128(2xdh), 512] accumulated over chunks
                for w_tile, dst, bcol0 in ((wq_t, qt, 0), (wk_t, kt, NPAIR)):
                    for p in range(NPAIR):
                        for n in range(NCH):
                            ps = psA.tile([128, 512], f32)
                            for c in range(CHUNKS):
                                nc.tensor.matmul(
                                    ps[:],
                                    w_tile[:, c, 128 * p:128 * p + 128],
                                    xtr[:, c, 512 * n:512 * n + 512],
                                    start=(c == 0), stop=(c == CHUNKS - 1))
                            # evacuate + add per-partition bias (dh rows)
                            nc.scalar.activation(
                                dst[:, p, 512 * n:512 * n + 512], ps[:],
                                AF.Identity,
                                bias=bias_t[:, bcol0 + p:bcol0 + p + 1])

                # V: psum[128(s), 256] accumulated over chunks
                for sb in range(SBLK):
                    ps = psV.tile([128, HPC * DH], f32)
                    for c in range(CHUNKS):
                        nc.tensor.matmul(
                            ps[:],
                            xtr[:, c, 128 * sb:128 * sb + 128],
                            wv_t[:, c, :],
                            start=(c == 0), stop=(c == CHUNKS - 1))
                    nc.vector.tensor_tensor(
                        vsb[:, sb, :, 0:DH],
                        ps[:].rearrange("p (h d) -> p h d", h=HPC),
                        bvb_t[:].rearrange("p (h d) -> p h d", h=HPC),
                        op=OP.add)
                if debug:
                    nc.sync.dma_start(qt_d[:], qt[:].bitcast(f32))
                    nc.sync.dma_start(kt_d[:], kt[:].bitcast(f32))
                    nc.sync.dma_start(vsb_d[:], vsb[:].bitcast(f32))

            # ------------- Phase B + fused C, per q-chunk -------------
            with (
                tc.tile_pool(name="pt", bufs=6) as ptp,
                tc.tile_pool(name="srow", bufs=2) as srowp,
                tc.tile_pool(name="rcp", bufs=2) as rcpp,
                tc.tile_pool(name="todd", bufs=2) as toddp,
                tc.tile_pool(name="ob", bufs=4) as obp,
                tc.tile_pool(name="psS", bufs=2, space="PSUM") as psS,
                tc.tile_pool(name="psAt", bufs=1, space="PSUM") as psAt,
                tc.tile_pool(name="psO", bufs=2, space="PSUM") as psO,
            ):
                for ch in range(NCH):
                    js = list(range(4 * ch, SBLK))
                    psa = [psAt.tile([DH + 1, 512], f32, tag=f"psa{h}",
                                     name=f"psa{h}")
                           for h in range(HPC)]
                    for idx, j in enumerate(js):
                        d = j - 4 * ch
                        W = 128 * (d + 1) if d < 4 else 512
                        pts = []
                        for h in range(HPC):
                            pair, half = h // 2, h % 2
                            r0 = 64 * half
                            pss = psS.tile([128, 512], f32, tag="pss",
                                           name="pss")
                            nc.tensor.matmul(
                                pss[:, 0:W],
                                kt[r0:r0 + 64, pair, 128 * j:128 * j + 128],
                                qt[r0:r0 + 64, pair,
                                   512 * ch:512 * ch + W],
                                start=True, stop=True)
                            pt = ptp.tile([128, 512], f32r)
                            nc.scalar.activation(pt[:, 0:W], pss[:, 0:W],
                                                 AF.Exp, scale=0.125)
                            if d < 4:
                                # zero-fill where k <= q (inverted causal)
                                nc.gpsimd.affine_select(
                                    pt[:, 0:W], pt[:, 0:W],
                                    pattern=[[-1, W]],
                                    base=128 * j - 512 * ch,
                                    channel_multiplier=1,
                                    compare_op=OP.is_gt,
                                    fill=0.0)
                            pts.append(pt)
                        last = (idx == len(js) - 1) and ch < 3
                        for h in range(HPC):
                            nc.tensor.matmul(
                                psa[h][:, 0:W], vsb[:, j, h, :],
                                pts[h][:, 0:W],
                                start=(idx == 0), stop=last)
                    if ch == 3:
                        # last global row q=2047: uniform over ALL keys.
                        # Column 511 is all-zero after masking; accumulate
                        # sum_k V[k] and the count 2048 via [0|1] columns.
                        for j in range(SBLK):
                            for h in range(HPC):
                                nc.tensor.matmul(
                                    psa[h][:, 510:512],
                                    vsb[:, j, h, :], ones2[:],
                                    start=False,
                                    stop=(j == SBLK - 1))
                    # normalize: attn^T rows / sums row.  Broadcast the
                    # sums row via a K=1 ones matmul, 64-lane reciprocal,
                    # then multiply.  Odd heads go through a SBUF tile and
                    # a partition-shifting DMA into rows 64:128 of the
                    # pair tile so phase C can contract K=128.
                    atn = atnp.tile([128, NPAIR, 512], f32r)
                    for h in range(HPC):
                        pair, half = h // 2, h % 2
                        srow = srowp.tile([DH + 1, 512], f32r)
                        nc.scalar.copy(srow[DH:DH + 1, :],
                                       psa[h][DH:DH + 1, :])
                        bcs = psS.tile([64, 512], f32, tag="pss",
                                       name="bcs")
                        nc.tensor.matmul(bcs[:],
                                         onesrow[DH:DH + 1, :],
                                         srow[DH:DH + 1, :],
                                         start=True, stop=True)
                        rcp = rcpp.tile([64, 512], f32)
                        nc.vector.reciprocal_approx_fast(rcp[:], bcs[:])
                        if half == 0:
                            nc.vector.tensor_tensor(
                                atn[0:64, pair, :], psa[h][0:DH, :],
                                rcp[:], op=OP.mult)
                        else:
                            todd = toddp.tile([64, 512], f32r)
                            nc.vector.tensor_tensor(
                                todd[:], psa[h][0:DH, :], rcp[:],
                                op=OP.mult)
                            nc.sync.dma_start(atn[64:128, pair, :],
                                              todd[:])
                    if debug:
                        for h in range(HPC):
                            dcp = srowp.tile([DH + 1, 512], f32,
                                             name="dcp", tag="dcp")
                            nc.vector.tensor_copy(dcp[:], psa[h][:])
                            nc.sync.dma_start(psa_d[:, ch, h, :], dcp[:])
                        nc.sync.dma_start(atn_d[:, ch, :, :],
                                          atn[:].bitcast(f32))

                    # fused phase C for this chunk's 4 s-blocks
                    for k in range(4):
                        sb = 4 * ch + k
                        for n in range(2):
                            ps = psO.tile([128, 512], f32)
                            for p in range(NPAIR):
                                nc.tensor.matmul(
                                    ps[:],
                                    atn[:, p, 128 * k:128 * k + 128],
                                    wo_t[:, p, 512 * n:512 * n + 512],
                                    start=(p == 0), stop=(p == NPAIR - 1))
                            ob = obp.tile([128, 512], f32)
                            nc.vector.tensor_copy(ob[:], ps[:])
                            nc.sync.dma_start(
                                out[128 * sb:128 * sb + 128,
                                    512 * n:512 * n + 512], ob[:])

    nc.finalize()
    return nc


def _prep_in_maps(inputs, Wq, bq, Wk, bk, Wv, bv, Wo, bo):
    in_maps = []
    xTs = [np.ascontiguousarray(inputs[b].T) for b in range(B)]
    for core in range(NCORES):
        b = core // (NCORES // B)
        g = core % (NCORES // B)
        cols = slice(g * HPC * DH, (g + 1) * HPC * DH)
        bq_c = bq[cols].reshape(NPAIR, 128).T          # [128, 2]
        bk_c = bk[cols].reshape(NPAIR, 128).T
        bqk_c = np.ascontiguousarray(
            np.concatenate([bq_c, bk_c], axis=1), dtype=np.float32)
        bvb_c = np.ascontiguousarray(
            np.broadcast_to(bv[cols][None, :], (128, HPC * DH)),
            dtype=np.float32)
        in_maps.append({
            "xT": xTs[b],
            "wq": np.ascontiguousarray(Wq[:, cols]),
            "wk": np.ascontiguousarray(Wk[:, cols]),
            "wv": np.ascontiguousarray(Wv[:, cols]),
            "wo": np.ascontiguousarray(Wo[cols, :]),
            "bqk": bqk_c,
            "bvb": bvb_c,
        })
    return in_maps


def kernel(inputs, Wq, bq, Wk, bk, Wv, bv, Wo, bo, _want_results=False,
           **_run_kwargs):
    from concourse.bass_utils import run_bass_kernel_spmd

    inputs = np.asarray(inputs, dtype=np.float32)
    Wq, bq = np.asarray(Wq, np.float32), np.asarray(bq, np.float32)
    Wk, bk = np.asarray(Wk, np.float32), np.asarray(bk, np.float32)
    Wv, bv = np.asarray(Wv, np.float32), np.asarray(bv, np.float32)
    Wo, bo = np.asarray(Wo, np.float32), np.asarray(bo, np.float32)

    if "nc" not in _CACHE:
        _CACHE["nc"] = _build_nc()
    nc = _CACHE["nc"]

    in_maps = _prep_in_maps(inputs, Wq, bq, Wk, bk, Wv, bv, Wo, bo)
    res = run_bass_kernel_spmd(nc, in_maps, core_ids=list(range(NCORES)),
                               **_run_kwargs)

    out = np.zeros((B, S, D), dtype=np.float32)
    for core in range(NCORES):
        b = core // (NCORES // B)
        out[b] += res.results[core]["out"]
    out += bo[None, None, :]
    if _want_results:
        return out, res
    return out


# revision 12
# speedup vs baseline: 1.5091x; 1.2369x over previous
"""Causal self-attention (with the reference's inverted mask) on 8 TRN2
NeuronCores.

Problem (hardcoded): B=2, S=2048, D=1024, H=16 heads, head_dim=64, fp32.
  q/k/v = x @ W* + b*;  score = q k^T / 8;  score += tril(ones)*(-1e9)
  (inverted causal mask: the LOWER triangle incl. diagonal is masked, so
  softmax attends strictly to k > q; row q=S-1 is fully masked and its
  softmax is exactly uniform, since all its masked inputs round to exactly
  -1e9 in fp32);  out = softmax(score) @ v @ Wo + bo.

Sharding: core c handles batch b = c//4 and heads [4*(c%4), 4*(c%4)+4).
Each core computes a partial output (its 4 heads' slice of attn @ Wo);
the host sums 4 partials per batch and adds bo.

Per-core kernel (all matmuls in float32r — TF32-like, ~1.5e-4 rel err,
full PE speed at N>=256):
  Phase A: QT/KT = W^T x^T in [dh, s] layout (head pairs packed to 128
    partitions), V in [s, dh] layout with an extra ones column per head
    ([V | 1]) so one matmul later yields both the attn numerator and the
    softmax denominator.
  Phase B (per q-chunk of 512): scores computed TRANSPOSED,
    s^T[k, q] = K^T Q per (head, k-block j), so softmax needs no
    max-subtraction and no transposes of the probability matrix:
    p^T = exp(s^T/8) (safe: |s|/8 is O(10); masked entries are skipped
    or zero-filled, matching the reference where exp(-1e9 - max)
    underflows to exactly 0).  Only k-blocks j >= 4c are active, and for
    diagonal blocks j = 4c+d only the first 128(d+1) q-columns can be
    unmasked, so score/exp/select/attn all narrow to that width; the
    in-block triangle gets an affine_select zero-fill where k <= q.
    attn^T[dh|sum, q] accumulates matmul([V|1], p^T) over j in PSUM.
    The globally-masked last row (q=2047) is exactly uniform attention
    over all 2048 keys; reproduced by N=2 [0|1]-column matmuls over all
    16 k-blocks into columns 510:512 (adding zero to 510).
    Normalization: broadcast the sums row to 64 partitions with a K=1
    ones matmul, then a 64-lane reciprocal and multiply (a 1-lane
    reciprocal measured 3.3us; this path is ~10x cheaper).
  Phase C (fused per q-chunk): out_partial[s-blocks of this chunk, :] =
    attn^T.T @ Wo-rows, heads packed in pairs so the contraction runs
    K=128 (odd heads DMA-shifted to partitions 64:128).
"""

import numpy as np

B, S, D, H, DH = 2, 2048, 1024, 16, 64
HPC = 4                 # heads per core
NCORES = 8
NPAIR = HPC // 2        # head pairs per core (2)
SBLK = S // 128         # 16 s/k blocks
NCH = S // 512          # 4 q-chunks of 512
CHUNKS = D // 128       # 8 contraction chunks of the model dim

_CACHE = {}


def _build_nc(debug=False):
    import concourse.mybir as mybir
    from concourse import bacc, tile

    f32 = mybir.dt.float32
    f32r = mybir.dt.float32r
    AF = mybir.ActivationFunctionType
    OP = mybir.AluOpType

    nc = bacc.Bacc("TRN2", target_bir_lowering=False)

    xT = nc.dram_tensor("xT", [D, S], f32, kind="ExternalInput")
    wq = nc.dram_tensor("wq", [D, HPC * DH], f32, kind="ExternalInput")
    wk = nc.dram_tensor("wk", [D, HPC * DH], f32, kind="ExternalInput")
    wv = nc.dram_tensor("wv", [D, HPC * DH], f32, kind="ExternalInput")
    wo = nc.dram_tensor("wo", [HPC * DH, D], f32, kind="ExternalInput")
    # per-pair q/k biases: [128, 4] cols = (q pair0, q pair1, k pair0, k pair1)
    bqk = nc.dram_tensor("bqk", [128, 2 * NPAIR], f32, kind="ExternalInput")
    # bv broadcast to all partitions host-side: [128, 256]
    bvb = nc.dram_tensor("bvb", [128, HPC * DH], f32, kind="ExternalInput")
    out = nc.dram_tensor("out", [S, D], f32, kind="ExternalOutput")
    if debug:
        qt_d = nc.dram_tensor("qt_d", [128, NPAIR, S], f32,
                              kind="ExternalOutput")
        kt_d = nc.dram_tensor("kt_d", [128, NPAIR, S], f32,
                              kind="ExternalOutput")
        vsb_d = nc.dram_tensor("vsb_d", [128, SBLK, HPC, DH + 1], f32,
                               kind="ExternalOutput")
        atn_d = nc.dram_tensor("atn_d", [128, NCH, NPAIR, 512], f32,
                               kind="ExternalOutput")
        psa_d = nc.dram_tensor("psa_d", [DH + 1, NCH, HPC, 512], f32,
                               kind="ExternalOutput")

    with tile.TileContext(nc) as tc:
        with (
            tc.tile_pool(name="pers", bufs=1) as pers,
            tc.tile_pool(name="atnp", bufs=2) as atnp,
            tc.tile_pool(name="misc", bufs=1) as misc,
        ):
            # Q^T head pairs, two variants with the other head's rows
            # zeroed so score matmuls can contract K=128 (K=64 f32r
            # matmuls never register HAM activity and run at 1.2 GHz)
            qze = pers.tile([128, NPAIR, S], f32r)        # odd rows zero
            qzo = pers.tile([128, NPAIR, S], f32r)        # even rows zero
            kt = pers.tile([128, NPAIR, S], f32r)         # K^T head pairs
            vsb = pers.tile([128, SBLK, HPC, DH + 1], f32r)  # [V | 1]
            wo_t = pers.tile([128, NPAIR, D], f32r)       # Wo head pairs
            ones2 = misc.tile([128, 2], f32r)   # [0 | 1] columns
            onef = misc.tile([128, 2], f32)
            onesrow = misc.tile([DH + 1, DH], f32r)  # row 64 = ones
            bias_t = misc.tile([128, 2 * NPAIR], f32)
            bvb_t = misc.tile([128, HPC * DH], f32)

            nc.sync.dma_start(bias_t[:], bqk[:])
            nc.sync.dma_start(bvb_t[:], bvb[:])
            nc.gpsimd.memset(onef[:, 0:1], 0.0)
            nc.gpsimd.memset(onef[:, 1:2], 1.0)
            nc.vector.tensor_copy(ones2[:], onef[:])  # rounded f32r [0|1]
            nc.vector.tensor_copy(
                onesrow[DH:DH + 1, :],
                onef[DH:DH + 1, 1:2].to_broadcast((1, DH)))
            # ones column of [V|1] for every (sblk, head)
            nc.vector.tensor_copy(
                vsb[:, :, :, DH:DH + 1],
                onef[:, 1:2].to_broadcast((128, SBLK, HPC, 1)))
            nc.vector.tensor_copy(
                qze[64:128, :, :],
                onef[64:128, 0:1].to_broadcast((64, NPAIR, S)))
            nc.vector.tensor_copy(
                qzo[0:64, :, :],
                onef[0:64, 0:1].to_broadcast((64, NPAIR, S)))

            # ---------------- Phase A: projections ----------------
            with (
                tc.tile_pool(name="stw", bufs=1) as stwp,
                tc.tile_pool(name="stx", bufs=2) as stxp,
                tc.tile_pool(name="wts", bufs=1) as wts,
                tc.tile_pool(name="psA", bufs=4, space="PSUM") as psA,
                tc.tile_pool(name="psV", bufs=2, space="PSUM") as psV,
            ):
                xtr = wts.tile([128, CHUNKS, S], f32r)
                wq_t = wts.tile([128, CHUNKS, HPC * DH], f32r, tag="wq")
                wk_t = wts.tile([128, CHUNKS, HPC * DH], f32r, tag="wk")
                wv_t = wts.tile([128, CHUNKS, HPC * DH], f32r, tag="wv")

                for w_dram, w_tile in ((wq, wq_t), (wk, wk_t), (wv, wv_t)):
                    st = stwp.tile([128, CHUNKS, HPC * DH], f32, tag="stw")
                    nc.sync.dma_start(
                        st[:], w_dram.rearrange("(c p) m -> p c m", p=128))
                    nc.vector.tensor_copy(w_tile[:], st[:])
                # Wo pairs: rows of pair p = wo[128p : 128p+128]
                wo_r = wo.rearrange("(p r) n -> p r n", r=128)
                for p in range(NPAIR):
                    st = stwp.tile([128, D], f32, tag="stw")
                    nc.sync.dma_start(st[:], wo_r[p])
                    nc.vector.tensor_copy(wo_t[:, p, :], st[:])

                xT_r = xT.rearrange("(c p) s -> c p s", p=128)
                for c in range(CHUNKS):
                    for half in range(2):
                        st = stxp.tile([128, S // 2], f32, tag="stx")
                        sl = slice(half * (S // 2), (half + 1) * (S // 2))
                        nc.sync.dma_start(st[:], xT_r[c][:, sl])
                        nc.vector.tensor_copy(xtr[:, c, sl], st[:])

                # QT / KT: psum[128(2xdh), 512] accumulated over chunks
                for w_tile, dsts, bcol0 in (
                        (wq_t, "q", 0), (wk_t, "k", NPAIR)):
                    for p in range(NPAIR):
                        for n in range(NCH):
                            ps = psA.tile([128, 512], f32)
                            for c in range(CHUNKS):
                                nc.tensor.matmul(
                                    ps[:],
                                    w_tile[:, c, 128 * p:128 * p + 128],
                                    xtr[:, c, 512 * n:512 * n + 512],
                                    start=(c == 0), stop=(c == CHUNKS - 1))
                            # evacuate + add per-partition bias (dh rows)
                            sl = slice(512 * n, 512 * n + 512)
                            bias = bias_t[:, bcol0 + p:bcol0 + p + 1]
                            if dsts == "k":
                                nc.scalar.activation(
                                    kt[:, p, sl], ps[:], AF.Identity,
                                    bias=bias)
                            else:
                                nc.scalar.activation(
                                    qze[0:64, p, sl], ps[0:64, :],
                                    AF.Identity, bias=bias[0:64, :])
                                nc.scalar.activation(
                                    qzo[64:128, p, sl], ps[64:128, :],
                                    AF.Identity, bias=bias[64:128, :])

                # V: psum[128(s), 256] accumulated over chunks
                for sb in range(SBLK):
                    ps = psV.tile([128, HPC * DH], f32)
                    for c in range(CHUNKS):
                        nc.tensor.matmul(
                            ps[:],
                            xtr[:, c, 128 * sb:128 * sb + 128],
                            wv_t[:, c, :],
                            start=(c == 0), stop=(c == CHUNKS - 1))
                    nc.vector.tensor_tensor(
                        vsb[:, sb, :, 0:DH],
                        ps[:].rearrange("p (h d) -> p h d", h=HPC),
                        bvb_t[:].rearrange("p (h d) -> p h d", h=HPC),
                        op=OP.add)
                if debug:
                    nc.sync.dma_start(qt_d[:], qt[:].bitcast(f32))
                    nc.sync.dma_start(kt_d[:], kt[:].bitcast(f32))
                    nc.sync.dma_start(vsb_d[:], vsb[:].bitcast(f32))

            # ------------- Phase B + fused C, per q-chunk -------------
            with (
                tc.tile_pool(name="pt", bufs=6) as ptp,
                tc.tile_pool(name="srow", bufs=2) as srowp,
                tc.tile_pool(name="rcp", bufs=2) as rcpp,
                tc.tile_pool(name="todd", bufs=2) as toddp,
                tc.tile_pool(name="ob", bufs=4) as obp,
                tc.tile_pool(name="psS", bufs=2, space="PSUM") as psS,
                tc.tile_pool(name="psAt", bufs=1, space="PSUM") as psAt,
                tc.tile_pool(name="psO", bufs=2, space="PSUM") as psO,
            ):
                for ch in range(NCH):
                    js = list(range(4 * ch, SBLK))
                    psa = [psAt.tile([DH + 1, 512], f32, tag=f"psa{h}",
                                     name=f"psa{h}")
                           for h in range(HPC)]
                    for idx, j in enumerate(js):
                        d = j - 4 * ch
                        W = 128 * (d + 1) if d < 4 else 512
                        pts = []
                        for h in range(HPC):
                            pair, half = h // 2, h % 2
                            qz = qze if half == 0 else qzo
                            pss = psS.tile([128, 512], f32, tag="pss",
                                           name="pss")
                            nc.tensor.matmul(
                                pss[:, 0:W],
                                kt[:, pair, 128 * j:128 * j + 128],
                                qz[:, pair, 512 * ch:512 * ch + W],
                                start=True, stop=True)
                            pt = ptp.tile([128, 512], f32r)
                            nc.scalar.activation(pt[:, 0:W], pss[:, 0:W],
                                                 AF.Exp, scale=0.125)
                            if d < 4:
                                # zero-fill where k <= q (inverted causal)
                                nc.gpsimd.affine_select(
                                    pt[:, 0:W], pt[:, 0:W],
                                    pattern=[[-1, W]],
                                    base=128 * j - 512 * ch,
                                    channel_multiplier=1,
                                    compare_op=OP.is_gt,
                                    fill=0.0)
                            pts.append(pt)
                        last = (idx == len(js) - 1) and ch < 3
                        for h in range(HPC):
                            nc.tensor.matmul(
                                psa[h][:, 0:W], vsb[:, j, h, :],
                                pts[h][:, 0:W],
                                start=(idx == 0), stop=last)
                    if ch == 3:
                        # last global row q=2047: uniform over ALL keys.
                        # Column 511 is all-zero after masking; accumulate
                        # sum_k V[k] and the count 2048 via [0|1] columns.
                        for j in range(SBLK):
                            for h in range(HPC):
                                nc.tensor.matmul(
                                    psa[h][:, 510:512],
                                    vsb[:, j, h, :], ones2[:],
                                    start=False,
                                    stop=(j == SBLK - 1))
                    # normalize: attn^T rows / sums row.  Broadcast the
                    # sums row via a K=1 ones matmul, 64-lane reciprocal,
                    # then multiply.  Odd heads go through a SBUF tile and
                    # a partition-shifting DMA into rows 64:128 of the
                    # pair tile so phase C can contract K=128.
                    atn = atnp.tile([128, NPAIR, 512], f32r)
                    for h in range(HPC):
                        pair, half = h // 2, h % 2
                        srow = srowp.tile([DH + 1, 512], f32r)
                        nc.scalar.copy(srow[DH:DH + 1, :],
                                       psa[h][DH:DH + 1, :])
                        bcs = psS.tile([64, 512], f32, tag="pss",
                                       name="bcs")
                        nc.tensor.matmul(bcs[:],
                                         onesrow[DH:DH + 1, :],
                                         srow[DH:DH + 1, :],
                                         start=True, stop=True)
                        rcp = rcpp.tile([64, 512], f32)
                        nc.vector.reciprocal_approx_fast(rcp[:], bcs[:])
                        if half == 0:
                            nc.vector.tensor_tensor(
                                atn[0:64, pair, :], psa[h][0:DH, :],
                                rcp[:], op=OP.mult)
                        else:
                            todd = toddp.tile([64, 512], f32r)
                            nc.vector.tensor_tensor(
                                todd[:], psa[h][0:DH, :], rcp[:],
                                op=OP.mult)
                            nc.sync.dma_start(atn[64:128, pair, :],
                                              todd[:])
                    if debug:
                        for h in range(HPC):
                            dcp = srowp.tile([DH + 1, 512], f32,
                                             name="dcp", tag="dcp")
                            nc.vector.tensor_copy(dcp[:], psa[h][:])
                            nc.sync.dma_start(psa_d[:, ch, h, :], dcp[:])
                        nc.sync.dma_start(atn_d[:, ch, :, :],
                                          atn[:].bitcast(f32))

                    # fused phase C for this chunk's 4 s-blocks
                    for k in range(4):
                        sb = 4 * ch + k
                        for n in range(2):
                            ps = psO.tile([128, 512], f32)
                            for p in range(NPAIR):
                                nc.tensor.matmul(
                                    ps[:],
                                    atn[:, p, 128 * k:128 * k + 128],
                                    wo_t[:, p, 512 * n:512 * n + 512],
                                    start=(p == 0), stop=(p == NPAIR - 1))
                            ob = obp.tile([128, 512], f32)
                            nc.vector.tensor_copy(ob[:], ps[:])
                            nc.sync.dma_start(
                                out[128 * sb:128 * sb + 128,
                                    512 * n:512 * n + 512], ob[:])

    nc.finalize()
    return nc


def _prep_in_maps(inputs, Wq, bq, Wk, bk, Wv, bv, Wo, bo):
    in_maps = []
    xTs = [np.ascontiguousarray(inputs[b].T) for b in range(B)]
    for core in range(NCORES):
        b = core // (NCORES // B)
        g = core % (NCORES // B)
        cols = slice(g * HPC * DH, (g + 1) * HPC * DH)
        bq_c = bq[cols].reshape(NPAIR, 128).T          # [128, 2]
        bk_c = bk[cols].reshape(NPAIR, 128).T
        bqk_c = np.ascontiguousarray(
            np.concatenate([bq_c, bk_c], axis=1), dtype=np.float32)
        bvb_c = np.ascontiguousarray(
            np.broadcast_to(bv[cols][None, :], (128, HPC * DH)),
            dtype=np.float32)
        in_maps.append({
            "xT": xTs[b],
            "wq": np.ascontiguousarray(Wq[:, cols]),
            "wk": np.ascontiguousarray(Wk[:, cols]),
            "wv": np.ascontiguousarray(Wv[:, cols]),
            "wo": np.ascontiguousarray(Wo[cols, :]),
            "bqk": bqk_c,
            "bvb": bvb_c,
        })
    return in_maps


def kernel(inputs, Wq, bq, Wk, bk, Wv, bv, Wo, bo, _want_results=False,
           **_run_kwargs):
    from concourse.bass_utils import run_bass_kernel_spmd

    inputs = np.asarray(inputs, dtype=np.float32)
    Wq, bq = np.asarray(Wq, np.float32), np.asarray(bq, np.float32)
    Wk, bk = np.asarray(Wk, np.float32), np.asarray(bk, np.float32)
    Wv, bv = np.asarray(Wv, np.float32), np.asarray(bv, np.float32)
    Wo, bo = np.asarray(Wo, np.float32), np.asarray(bo, np.float32)

    if "nc" not in _CACHE:
        _CACHE["nc"] = _build_nc()
    nc = _CACHE["nc"]

    in_maps = _prep_in_maps(inputs, Wq, bq, Wk, bk, Wv, bv, Wo, bo)
    res = run_bass_kernel_spmd(nc, in_maps, core_ids=list(range(NCORES)),
                               **_run_kwargs)

    out = np.zeros((B, S, D), dtype=np.float32)
    for core in range(NCORES):
        b = core // (NCORES // B)
        out[b] += res.results[core]["out"]
    out += bo[None, None, :]
    if _want_results:
        return out, res
    return out


# revision 13
# speedup vs baseline: 1.5202x; 1.0074x over previous
"""Causal self-attention (with the reference's inverted mask) on 8 TRN2
NeuronCores.

Problem (hardcoded): B=2, S=2048, D=1024, H=16 heads, head_dim=64, fp32.
  q/k/v = x @ W* + b*;  score = q k^T / 8;  score += tril(ones)*(-1e9)
  (inverted causal mask: the LOWER triangle incl. diagonal is masked, so
  softmax attends strictly to k > q; row q=S-1 is fully masked and its
  softmax is exactly uniform, since all its masked inputs round to exactly
  -1e9 in fp32);  out = softmax(score) @ v @ Wo + bo.

Sharding: core c handles batch b = c//4 and heads [4*(c%4), 4*(c%4)+4).
Each core computes a partial output (its 4 heads' slice of attn @ Wo);
the host sums 4 partials per batch and adds bo.

Per-core kernel (all matmuls in float32r — TF32-like, ~1.5e-4 rel err,
full PE speed at N>=256):
  Phase A: QT/KT = W^T x^T in [dh, s] layout (head pairs packed to 128
    partitions), V in [s, dh] layout with an extra ones column per head
    ([V | 1]) so one matmul later yields both the attn numerator and the
    softmax denominator.
  Phase B (per q-chunk of 512): scores computed TRANSPOSED,
    s^T[k, q] = K^T Q per (head, k-block j), so softmax needs no
    max-subtraction and no transposes of the probability matrix:
    p^T = exp(s^T/8) (safe: |s|/8 is O(10); masked entries are skipped
    or zero-filled, matching the reference where exp(-1e9 - max)
    underflows to exactly 0).  Only k-blocks j >= 4c are active, and for
    diagonal blocks j = 4c+d only the first 128(d+1) q-columns can be
    unmasked, so score/exp/select/attn all narrow to that width; the
    in-block triangle gets an affine_select zero-fill where k <= q.
    attn^T[dh|sum, q] accumulates matmul([V|1], p^T) over j in PSUM.
    The globally-masked last row (q=2047) is exactly uniform attention
    over all 2048 keys; reproduced by N=2 [0|1]-column matmuls over all
    16 k-blocks into columns 510:512 (adding zero to 510).
    Normalization: broadcast the sums row to 64 partitions with a K=1
    ones matmul, then a 64-lane reciprocal and multiply (a 1-lane
    reciprocal measured 3.3us; this path is ~10x cheaper).
  Phase C (fused per q-chunk): out_partial[s-blocks of this chunk, :] =
    attn^T.T @ Wo-rows, heads packed in pairs so the contraction runs
    K=128 (odd heads DMA-shifted to partitions 64:128).
"""

import numpy as np

B, S, D, H, DH = 2, 2048, 1024, 16, 64
HPC = 4                 # heads per core
NCORES = 8
NPAIR = HPC // 2        # head pairs per core (2)
SBLK = S // 128         # 16 s/k blocks
NCH = S // 512          # 4 q-chunks of 512
CHUNKS = D // 128       # 8 contraction chunks of the model dim

_CACHE = {}


def _build_nc(debug=False):
    import concourse.mybir as mybir
    from concourse import bacc, tile

    f32 = mybir.dt.float32
    f32r = mybir.dt.float32r
    AF = mybir.ActivationFunctionType
    OP = mybir.AluOpType

    nc = bacc.Bacc("TRN2", target_bir_lowering=False)

    xT = nc.dram_tensor("xT", [D, S], f32, kind="ExternalInput")
    wq = nc.dram_tensor("wq", [D, HPC * DH], f32, kind="ExternalInput")
    wk = nc.dram_tensor("wk", [D, HPC * DH], f32, kind="ExternalInput")
    wv = nc.dram_tensor("wv", [D, HPC * DH], f32, kind="ExternalInput")
    wo = nc.dram_tensor("wo", [HPC * DH, D], f32, kind="ExternalInput")
    # per-pair q/k biases: [128, 4] cols = (q pair0, q pair1, k pair0, k pair1)
    bqk = nc.dram_tensor("bqk", [128, 2 * NPAIR], f32, kind="ExternalInput")
    # bv broadcast to all partitions host-side: [128, 256]
    bvb = nc.dram_tensor("bvb", [128, HPC * DH], f32, kind="ExternalInput")
    out = nc.dram_tensor("out", [S, D], f32, kind="ExternalOutput")
    if debug:
        qt_d = nc.dram_tensor("qt_d", [128, NPAIR, S], f32,
                              kind="ExternalOutput")
        kt_d = nc.dram_tensor("kt_d", [128, NPAIR, S], f32,
                              kind="ExternalOutput")
        vsb_d = nc.dram_tensor("vsb_d", [128, SBLK, HPC, DH + 1], f32,
                               kind="ExternalOutput")
        atn_d = nc.dram_tensor("atn_d", [128, NCH, NPAIR, 512], f32,
                               kind="ExternalOutput")
        psa_d = nc.dram_tensor("psa_d", [DH + 1, NCH, HPC, 512], f32,
                               kind="ExternalOutput")

    with tile.TileContext(nc) as tc:
        with (
            tc.tile_pool(name="pers", bufs=1) as pers,
            tc.tile_pool(name="atnp", bufs=2) as atnp,
            tc.tile_pool(name="misc", bufs=1) as misc,
        ):
            # Q^T head pairs, two variants with the other head's rows
            # zeroed so score matmuls can contract K=128 (K=64 f32r
            # matmuls never register HAM activity and run at 1.2 GHz)
            qze = pers.tile([128, NPAIR, S], f32r)        # odd rows zero
            qzo = pers.tile([128, NPAIR, S], f32r)        # even rows zero
            kt = pers.tile([128, NPAIR, S], f32r)         # K^T head pairs
            vsb = pers.tile([128, SBLK, HPC, DH + 1], f32r)  # [V | 1]
            wo_t = pers.tile([128, NPAIR, D], f32r)       # Wo head pairs
            ones2 = misc.tile([128, 2], f32r)   # [0 | 1] columns
            onef = misc.tile([128, 2], f32)
            onesrow = misc.tile([DH + 1, DH], f32r)  # row 64 = ones
            bias_t = misc.tile([128, 2 * NPAIR], f32)
            bvb_t = misc.tile([128, HPC * DH], f32)

            nc.sync.dma_start(bias_t[:], bqk[:])
            nc.sync.dma_start(bvb_t[:], bvb[:])
            nc.gpsimd.memset(onef[:, 0:1], 0.0)
            nc.gpsimd.memset(onef[:, 1:2], 1.0)
            nc.vector.tensor_copy(ones2[:], onef[:])  # rounded f32r [0|1]
            nc.vector.tensor_copy(
                onesrow[DH:DH + 1, :],
                onef[DH:DH + 1, 1:2].to_broadcast((1, DH)))
            # ones column of [V|1] for every (sblk, head)
            nc.vector.tensor_copy(
                vsb[:, :, :, DH:DH + 1],
                onef[:, 1:2].to_broadcast((128, SBLK, HPC, 1)))
            nc.vector.tensor_copy(
                qze[64:128, :, :],
                onef[64:128, 0:1].to_broadcast((64, NPAIR, S)))
            nc.vector.tensor_copy(
                qzo[0:64, :, :],
                onef[0:64, 0:1].to_broadcast((64, NPAIR, S)))

            # ---------------- Phase A: projections ----------------
            with (
                tc.tile_pool(name="stw", bufs=1) as stwp,
                tc.tile_pool(name="stx", bufs=2) as stxp,
                tc.tile_pool(name="wts", bufs=1) as wts,
                tc.tile_pool(name="psA", bufs=4, space="PSUM") as psA,
                tc.tile_pool(name="psV", bufs=2, space="PSUM") as psV,
            ):
                xtr = wts.tile([128, CHUNKS, S], f32r)
                wq_t = wts.tile([128, CHUNKS, HPC * DH], f32r, tag="wq")
                wk_t = wts.tile([128, CHUNKS, HPC * DH], f32r, tag="wk")
                wv_t = wts.tile([128, CHUNKS, HPC * DH], f32r, tag="wv")

                for w_dram, w_tile in ((wq, wq_t), (wk, wk_t), (wv, wv_t)):
                    st = stwp.tile([128, CHUNKS, HPC * DH], f32, tag="stw")
                    nc.scalar.dma_start(
                        st[:], w_dram.rearrange("(c p) m -> p c m", p=128))
                    nc.vector.tensor_copy(w_tile[:], st[:])
                # Wo pairs: rows of pair p = wo[128p : 128p+128]
                wo_r = wo.rearrange("(p r) n -> p r n", r=128)
                for p in range(NPAIR):
                    st = stwp.tile([128, D], f32, tag="stw")
                    nc.scalar.dma_start(st[:], wo_r[p])
                    nc.vector.tensor_copy(wo_t[:, p, :], st[:])

                xT_r = xT.rearrange("(c p) s -> c p s", p=128)
                for c in range(CHUNKS):
                    for half in range(2):
                        st = stxp.tile([128, S // 2], f32, tag="stx")
                        sl = slice(half * (S // 2), (half + 1) * (S // 2))
                        eng = nc.sync if half == 0 else nc.scalar
                        eng.dma_start(st[:], xT_r[c][:, sl])
                        nc.vector.tensor_copy(xtr[:, c, sl], st[:])

                # QT / KT: psum[128(2xdh), 512] accumulated over chunks
                for w_tile, dsts, bcol0 in (
                        (wq_t, "q", 0), (wk_t, "k", NPAIR)):
                    for p in range(NPAIR):
                        for n in range(NCH):
                            ps = psA.tile([128, 512], f32)
                            for c in range(CHUNKS):
                                nc.tensor.matmul(
                                    ps[:],
                                    w_tile[:, c, 128 * p:128 * p + 128],
                                    xtr[:, c, 512 * n:512 * n + 512],
                                    start=(c == 0), stop=(c == CHUNKS - 1))
                            # evacuate + add per-partition bias (dh rows)
                            sl = slice(512 * n, 512 * n + 512)
                            bias = bias_t[:, bcol0 + p:bcol0 + p + 1]
                            if dsts == "k":
                                nc.scalar.activation(
                                    kt[:, p, sl], ps[:], AF.Identity,
                                    bias=bias)
                            else:
                                nc.scalar.activation(
                                    qze[0:64, p, sl], ps[0:64, :],
                                    AF.Identity, bias=bias[0:64, :])
                                nc.scalar.activation(
                                    qzo[64:128, p, sl], ps[64:128, :],
                                    AF.Identity, bias=bias[64:128, :])

                # V: psum[128(s), 256] accumulated over chunks
                for sb in range(SBLK):
                    ps = psV.tile([128, HPC * DH], f32)
                    for c in range(CHUNKS):
                        nc.tensor.matmul(
                            ps[:],
                            xtr[:, c, 128 * sb:128 * sb + 128],
                            wv_t[:, c, :],
                            start=(c == 0), stop=(c == CHUNKS - 1))
                    nc.vector.tensor_tensor(
                        vsb[:, sb, :, 0:DH],
                        ps[:].rearrange("p (h d) -> p h d", h=HPC),
                        bvb_t[:].rearrange("p (h d) -> p h d", h=HPC),
                        op=OP.add)
                if debug:
                    nc.sync.dma_start(qt_d[:], qt[:].bitcast(f32))
                    nc.sync.dma_start(kt_d[:], kt[:].bitcast(f32))
                    nc.sync.dma_start(vsb_d[:], vsb[:].bitcast(f32))

            # ------------- Phase B + fused C, per q-chunk -------------
            with (
                tc.tile_pool(name="pt", bufs=6) as ptp,
                tc.tile_pool(name="srow", bufs=2) as srowp,
                tc.tile_pool(name="rcp", bufs=2) as rcpp,
                tc.tile_pool(name="todd", bufs=2) as toddp,
                tc.tile_pool(name="ob", bufs=4) as obp,
                tc.tile_pool(name="psS", bufs=2, space="PSUM") as psS,
                tc.tile_pool(name="psAt", bufs=1, space="PSUM") as psAt,
                tc.tile_pool(name="psO", bufs=2, space="PSUM") as psO,
            ):
                for ch in range(NCH):
                    js = list(range(4 * ch, SBLK))
                    psa = [psAt.tile([DH + 1, 512], f32, tag=f"psa{h}",
                                     name=f"psa{h}")
                           for h in range(HPC)]
                    for idx, j in enumerate(js):
                        d = j - 4 * ch
                        W = 128 * (d + 1) if d < 4 else 512
                        pts = []
                        for h in range(HPC):
                            pair, half = h // 2, h % 2
                            qz = qze if half == 0 else qzo
                            pss = psS.tile([128, 512], f32, tag="pss",
                                           name="pss")
                            nc.tensor.matmul(
                                pss[:, 0:W],
                                kt[:, pair, 128 * j:128 * j + 128],
                                qz[:, pair, 512 * ch:512 * ch + W],
                                start=True, stop=True)
                            pt = ptp.tile([128, 512], f32r)
                            nc.scalar.activation(pt[:, 0:W], pss[:, 0:W],
                                                 AF.Exp, scale=0.125)
                            if d < 4:
                                # zero-fill where k <= q (inverted causal)
                                nc.gpsimd.affine_select(
                                    pt[:, 0:W], pt[:, 0:W],
                                    pattern=[[-1, W]],
                                    base=128 * j - 512 * ch,
                                    channel_multiplier=1,
                                    compare_op=OP.is_gt,
                                    fill=0.0)
                            pts.append(pt)
                        last = (idx == len(js) - 1) and ch < 3
                        for h in range(HPC):
                            nc.tensor.matmul(
                                psa[h][:, 0:W], vsb[:, j, h, :],
                                pts[h][:, 0:W],
                                start=(idx == 0), stop=last)
                    if ch == 3:
                        # last global row q=2047: uniform over ALL keys.
                        # Column 511 is all-zero after masking; accumulate
                        # sum_k V[k] and the count 2048 via [0|1] columns.
                        for j in range(SBLK):
                            for h in range(HPC):
                                nc.tensor.matmul(
                                    psa[h][:, 510:512],
                                    vsb[:, j, h, :], ones2[:],
                                    start=False,
                                    stop=(j == SBLK - 1))
                    # normalize: attn^T rows / sums row.  Broadcast the
                    # sums row via a K=1 ones matmul, 64-lane reciprocal,
                    # then multiply.  Odd heads go through a SBUF tile and
                    # a partition-shifting DMA into rows 64:128 of the
                    # pair tile so phase C can contract K=128.
                    atn = atnp.tile([128, NPAIR, 512], f32r)
                    for h in range(HPC):
                        pair, half = h // 2, h % 2
                        srow = srowp.tile([DH + 1, 512], f32r)
                        nc.scalar.copy(srow[DH:DH + 1, :],
                                       psa[h][DH:DH + 1, :])
                        bcs = psS.tile([64, 512], f32, tag="pss",
                                       name="bcs")
                        nc.tensor.matmul(bcs[:],
                                         onesrow[DH:DH + 1, :],
                                         srow[DH:DH + 1, :],
                                         start=True, stop=True)
                        rcp = rcpp.tile([64, 512], f32)
                        nc.vector.reciprocal_approx_fast(rcp[:], bcs[:])
                        if half == 0:
                            nc.vector.tensor_tensor(
                                atn[0:64, pair, :], psa[h][0:DH, :],
                                rcp[:], op=OP.mult)
                        else:
                            todd = toddp.tile([64, 512], f32r)
                            nc.vector.tensor_tensor(
                                todd[:], psa[h][0:DH, :], rcp[:],
                                op=OP.mult)
                            nc.sync.dma_start(atn[64:128, pair, :],
                                              todd[:])
                    if debug:
                        for h in range(HPC):
                            dcp = srowp.tile([DH + 1, 512], f32,
                                             name="dcp", tag="dcp")
                            nc.vector.tensor_copy(dcp[:], psa[h][:])
                            nc.sync.dma_start(psa_d[:, ch, h, :], dcp[:])
                        nc.sync.dma_start(atn_d[:, ch, :, :],
                                          atn[:].bitcast(f32))

                    # fused phase C for this chunk's 4 s-blocks
                    for k in range(4):
                        sb = 4 * ch + k
                        for n in range(2):
                            ps = psO.tile([128, 512], f32)
                            for p in range(NPAIR):
                                nc.tensor.matmul(
                                    ps[:],
                                    atn[:, p, 128 * k:128 * k + 128],
                                    wo_t[:, p, 512 * n:512 * n + 512],
                                    start=(p == 0), stop=(p == NPAIR - 1))
                            ob = obp.tile([128, 512], f32)
                            nc.vector.tensor_copy(ob[:], ps[:])
                            nc.sync.dma_start(
                                out[128 * sb:128 * sb + 128,
                                    512 * n:512 * n + 512], ob[:])

    nc.finalize()
    return nc


def _prep_in_maps(inputs, Wq, bq, Wk, bk, Wv, bv, Wo, bo):
    in_maps = []
    xTs = [np.ascontiguousarray(inputs[b].T) for b in range(B)]
    for core in range(NCORES):
        b = core // (NCORES // B)
        g = core % (NCORES // B)
        cols = slice(g * HPC * DH, (g + 1) * HPC * DH)
        bq_c = bq[cols].reshape(NPAIR, 128).T          # [128, 2]
        bk_c = bk[cols].reshape(NPAIR, 128).T
        bqk_c = np.ascontiguousarray(
            np.concatenate([bq_c, bk_c], axis=1), dtype=np.float32)
        bvb_c = np.ascontiguousarray(
            np.broadcast_to(bv[cols][None, :], (128, HPC * DH)),
            dtype=np.float32)
        in_maps.append({
            "xT": xTs[b],
            "wq": np.ascontiguousarray(Wq[:, cols]),
            "wk": np.ascontiguousarray(Wk[:, cols]),
            "wv": np.ascontiguousarray(Wv[:, cols]),
            "wo": np.ascontiguousarray(Wo[cols, :]),
            "bqk": bqk_c,
            "bvb": bvb_c,
        })
    return in_maps


def kernel(inputs, Wq, bq, Wk, bk, Wv, bv, Wo, bo, _want_results=False,
           **_run_kwargs):
    from concourse.bass_utils import run_bass_kernel_spmd

    inputs = np.asarray(inputs, dtype=np.float32)
    Wq, bq = np.asarray(Wq, np.float32), np.asarray(bq, np.float32)
    Wk, bk = np.asarray(Wk, np.float32), np.asarray(bk, np.float32)
    Wv, bv = np.asarray(Wv, np.float32), np.asarray(bv, np.float32)
    Wo, bo = np.asarray(Wo, np.float32), np.asarray(bo, np.float32)

    if "nc" not in _CACHE:
        _CACHE["nc"] = _build_nc()
    nc = _CACHE["nc"]

    in_maps = _prep_in_maps(inputs, Wq, bq, Wk, bk, Wv, bv, Wo, bo)
    res = run_bass_kernel_spmd(nc, in_maps, core_ids=list(range(NCORES)),
                               **_run_kwargs)

    out = np.zeros((B, S, D), dtype=np.float32)
    for core in range(NCORES):
        b = core // (NCORES // B)
        out[b] += res.results[core]["out"]
    out += bo[None, None, :]
    if _want_results:
        return out, res
    return out
